# revision 6
# baseline (speedup 1.0000x reference)
"""Trainium2 Bass kernel for nn_ContrastSSIMLoss.

loss = mean_{b,h,w,s} | C_o(s,h,w) - C_s(s,h,w) |  over 120 shifts s=(i,j),
where C_img(s,h,w) = sum_c |img[c,h+5,w+5] - img[c,h+5+i,w+5+j]|,
output domain 246x246, B=16, C=3, H=256, w=5.

Strategy
- Pure data parallel: batch dim sharded 2-per-core across 8 NeuronCores.
- Half-shift trick: for s=(i,j) with i>0 or (i==0 and j>0), the map for -s is
  a translated copy of the map for s.  Compute F(y,x) = |A_o - A_s| once per
  half-shift on an extended domain, then two window sums:
    W1 = sum over y,x in [0,246)^2                   (contribution of s)
    W2 = sum over y in [-i,246-i), x in [-j,246-j)   (contribution of -s)
  W1/W2 share the column union; compute one union row-sum R (abs applied in
  the reduce) and subtract narrow edge sums.
- bf16 data path: tensor_tensor at 2x (dual-parity strips keep every shifted
  operand 4B-aligned).  Both images stacked in one strip tensor so sub/abs
  run as single big instructions.
- Engine split: DVE (subs/adds/reduces), ACT (abs), GPSIMD (one add per
  half-shift + small ops), DMA prologue.
- Layout: partition p = b*64+g (g in 0..62) owns image rows [4g, 4g+9)
  (4 output rows + 5 halo; half-shifts only look down/right).  Row-window
  validity of each partial sum is resolved host-side via per-
  (shift,window,row) slots; the global mean is computed on host in f64.
"""

import numpy as np

W = 5
H = 256
OUT = H - 2 * W          # 246
B_TOTAL, C = 16, 3
NCORES = 8
NB = B_TOTAL // NCORES   # 2 batches per core
RPB = 4                  # owned rows per block
SROWS = RPB + W          # 9 strip rows per partition
PADL = 8                 # left pad cols in strip (even => 4B-aligned bf16)
SCOLS = PADL + H + 8     # 272 padded strip row length
HS = [(i, j) for i in range(0, W + 1) for j in range(-W, W + 1)
      if i > 0 or (i == 0 and j > 0)]
assert len(HS) == 60
NSLOT = len(HS) * 2 * RPB  # 480

_COMPILED = None
LAST_RESULTS = None


def _build():
    import concourse.bass as bass
    import concourse.mybir as mybir
    from concourse import bacc, tile

    f32 = mybir.dt.float32
    bf16 = mybir.dt.bfloat16
    SUB = mybir.AluOpType.subtract
    ADD = mybir.AluOpType.add
    ABS = mybir.ActivationFunctionType.Abs
    AX = mybir.AxisListType.X

    nc = bacc.Bacc("TRN2", target_bir_lowering=False, debug=False,
                   num_devices=NCORES)

    imgs_dram = [
        nc.dram_tensor("orig", [NB, C, H, H], bf16, kind="ExternalInput"),
        nc.dram_tensor("simu", [NB, C, H, H], bf16, kind="ExternalInput"),
    ]
    out_dram = nc.dram_tensor("partials", [128, NSLOT], f32,
                              kind="ExternalOutput")

    with tile.TileContext(nc) as tc:
        with (
            tc.tile_pool(name="strips", bufs=1) as spool,
            tc.tile_pool(name="work", bufs=3) as wpool,
            tc.tile_pool(name="amaps", bufs=4) as apool,
            tc.tile_pool(name="red", bufs=4) as redpool,
            tc.tile_pool(name="res", bufs=1) as rpool,
        ):
            # both images stacked: [128, img, C, SROWS, SCOLS]
            sE = spool.tile([128, 2, C, SROWS, SCOLS], bf16, name="sE")
            sO = spool.tile([128, 2, C, SROWS, SCOLS], bf16, name="sO")
            slots = rpool.tile([128, NSLOT], f32, name="slots")

            # Load halo strips: partition b*64+g holds rows [4g, 4g+9),
            # data cols at [PADL, PADL+256).
            for im in range(2):
                src = imgs_dram[im]
                hsrc = src.tensor if hasattr(src, "tensor") else src
                for b in range(NB):
                    for cc in range(C):
                        coff = b * C * H * H + cc * H * H
                        dst = sE[b * 64:b * 64 + 62, im, cc, :, PADL:PADL + H]
                        nc.sync.dma_start(
                            out=dst,
                            in_=bass.AP(hsrc, coff,
                                        [[RPB * H, 62], [H, SROWS], [1, H]]))
                        # g = 62: only 8 rows (248..255)
                        dst2 = sE[b * 64 + 62:b * 64 + 63, im, cc, 0:8,
                                  PADL:PADL + H]
                        nc.sync.dma_start(
                            out=dst2,
                            in_=bass.AP(hsrc, coff + 62 * RPB * H,
                                        [[0, 1], [H, 8], [1, H]]))
            # odd-parity copy (data shifted one column right), on DVE
            nc.vector.tensor_copy(out=sO[:, :, :, :, PADL + 1:PADL + 1 + H],
                                  in_=sE[:, :, :, :, PADL:PADL + H])

            for k, (i, j) in enumerate(HS):
                d = wpool.tile([128, 2, C, RPB, H], bf16, tag="d",
                               name=f"d{k}")
                center = sE[:, :, :, 0:RPB, PADL:PADL + H]
                if j % 2 == 0:
                    shifted = sE[:, :, :, i:i + RPB, PADL + j:PADL + j + H]
                else:
                    shifted = sO[:, :, :, i:i + RPB,
                                 PADL + 1 + j:PADL + 1 + j + H]
                nc.vector.tensor_tensor(out=d[:], in0=center, in1=shifted,
                                        op=SUB)
                ad = wpool.tile([128, 2, C, RPB, H], bf16, tag="ad",
                                name=f"ad{k}")
                nc.scalar.activation(out=ad[:], in_=d[:], func=ABS)
                a01 = apool.tile([128, 2, RPB, H], bf16, tag="a01",
                                 name=f"a01{k}")
                nc.gpsimd.tensor_tensor(out=a01[:], in0=ad[:, :, 0],
                                        in1=ad[:, :, 1], op=ADD)
                a = apool.tile([128, 2, RPB, H], bf16, tag="a", name=f"a{k}")
                nc.vector.tensor_tensor(out=a[:], in0=a01[:], in1=ad[:, :, 2],
                                        op=ADD)
                f = wpool.tile([128, RPB, H], bf16, tag="f", name=f"f{k}")
                nc.vector.tensor_tensor(out=f[:], in0=a[:, 0], in1=a[:, 1],
                                        op=SUB)

                # column windows: W1 m in [5,251), W2 m in [5-j,251-j)
                # union reduce R minus narrow edges (all reduce |.|)
                base = k * 2 * RPB
                mlo, mhi = W - max(j, 0), W + OUT - min(j, 0)
                r = redpool.tile([128, RPB], f32, tag="r", name=f"r{k}")
                nc.vector.tensor_reduce(
                    out=r[:], in_=f[:, :, mlo:mhi], axis=AX, op=ADD,
                    apply_absolute_value=True)
                if j == 0:
                    nc.gpsimd.tensor_copy(out=slots[:, base:base + RPB],
                                          in_=r[:])
                    nc.gpsimd.tensor_copy(
                        out=slots[:, base + RPB:base + 2 * RPB], in_=r[:])
                else:
                    e1 = redpool.tile([128, RPB], f32, tag="e1", name=f"e1{k}")
                    e2 = redpool.tile([128, RPB], f32, tag="e2", name=f"e2{k}")
                    if j > 0:
                        # W1 = R - sum cols m in [5-j, 5); W2 = R - [251-j, 251)
                        s1 = f[:, :, W - j:W]
                        s2 = f[:, :, W + OUT - j:W + OUT]
                    else:
                        # j<0: W1 = R - [251, 251-j); W2 = R - [5, 5-j)
                        s1 = f[:, :, W + OUT:W + OUT - j]
                        s2 = f[:, :, W:W - j]
                    nc.vector.tensor_reduce(out=e1[:], in_=s1, axis=AX,
                                            op=ADD, apply_absolute_value=True)
                    nc.vector.tensor_reduce(out=e2[:], in_=s2, axis=AX,
                                            op=ADD, apply_absolute_value=True)
                    nc.vector.tensor_tensor(out=slots[:, base:base + RPB],
                                            in0=r[:], in1=e1[:], op=SUB)
                    nc.vector.tensor_tensor(
                        out=slots[:, base + RPB:base + 2 * RPB], in0=r[:],
                        in1=e2[:], op=SUB)

            nc.sync.dma_start(out=out_dram[:], in_=slots[:])

    nc.compile()
    return nc


def _slot_mask():
    """mask[p, slot] — True where the slot row belongs to the shift window."""
    mask = np.zeros((128, NSLOT), dtype=bool)
    for p in range(128):
        g = p % 64
        if g > 62:
            continue
        for k, (i, j) in enumerate(HS):
            for win in range(2):
                ylo, yhi = (0, OUT) if win == 0 else (-i, OUT - i)
                for r in range(RPB):
                    y = RPB * g - W + r
                    if ylo <= y < yhi:
                        mask[p, k * 2 * RPB + win * RPB + r] = True
    return mask


def _inject_ntff_hook():
    """Best-effort: register the axon NTFF profile hook so trace=True works."""
    import sys, types
    if "antenv.axon_hooks" in sys.modules:
        return
    try:
        import trn_agent_boot.trn_boot as tb
        hook = tb._ntff_profile_via_ctypes('/opt/axon/libaxon_pjrt.so')
    except Exception:
        return
    mod = types.ModuleType("antenv.axon_hooks")
    _h = [hook]
    mod.set_axon_ntff_profile_hook = lambda h: _h.__setitem__(0, h)
    mod.get_axon_ntff_profile_hook = lambda: _h[0]
    sys.modules["antenv.axon_hooks"] = mod


def kernel(original_image, simulated_image, window_size):
    global _COMPILED, LAST_RESULTS
    assert int(window_size) == W
    import ml_dtypes
    from concourse.bass_utils import run_bass_kernel_spmd

    _inject_ntff_hook()
    if _COMPILED is None:
        _COMPILED = _build()
    nc = _COMPILED

    orig = np.ascontiguousarray(
        np.asarray(original_image, dtype=np.float32).astype(ml_dtypes.bfloat16))
    simu = np.ascontiguousarray(
        np.asarray(simulated_image, dtype=np.float32).astype(ml_dtypes.bfloat16))
    in_maps = [
        {"orig": orig[c * NB:(c + 1) * NB], "simu": simu[c * NB:(c + 1) * NB]}
        for c in range(NCORES)
    ]
    res = run_bass_kernel_spmd(nc, in_maps, list(range(NCORES)))
    LAST_RESULTS = res

    mask = _slot_mask()
    total = 0.0
    for c in range(NCORES):
        s = res.results[c]["partials"]
        total += s[mask].sum(dtype=np.float64)
    loss = total / (B_TOTAL * len(HS) * 2 * OUT * OUT)
    return np.float32(loss)
